# revision 18
# baseline (speedup 1.0000x reference)
"""Grouped GEMM (MoE routing) Trainium2 kernel — w-stationary structure.

Expert-parallel across 8 NeuronCores with size-sorted slot assignment:
experts are sorted by token count and slot s on every core holds the
experts of size-rank [8s, 8s+8), so one SPMD program with per-slot
capacities cap_s = roundup4(max count in rank group) serves all cores.

Weights stream as fp8 E3M4 (scaled by 64, the 1/64 folded into the
bf16 x pack), halving the dominant HBM traffic. Each matmul loads a
dense [128,128] fp8 weight tile as the stationary operand (full-width
=> fast-weight-load) and streams the slot's token block [128, cap]
as the moving operand, so no PE cycle is spent on capacity padding
beyond cap_s. PSUM accumulates [128, cap] f32 over 20 K-chunks per
DOUT-chunk m; outputs evacuate to an m-major bf16 pack, one DMA per
slot. Output is produced transposed ([DOUT, tokens]); the host
unscrambles during the unshard (~1.2e-2 rel err from e3m4 weights).
"""
import ml_dtypes
import numpy as np

import concourse.bass as bass
import concourse.mybir as mybir
import concourse.tile as tile
from concourse import bacc
from concourse.bass_utils import run_bass_kernel_spmd

G, T, DIN, DOUT = 64, 8192, 2560, 1664
NCORES = 8
EPC = G // NCORES   # expert slots per core
KC = DIN // 128     # 20 contraction chunks
MC = DOUT // 128    # 13 dout chunks
WSCALE = 64.0       # fp8 e3m4 weight scale (folded back via x/WSCALE)

_cache = {}


def _build(caps):
    offs = np.concatenate([[0], np.cumsum(caps)]).astype(int)
    sumcap = int(offs[-1])
    nc = bacc.Bacc(trn_type="TRN2", debug=False)
    bf16 = mybir.dt.bfloat16
    f8 = mybir.dt.float8e3
    # xt: partition-major pack [128, KC*sumcap]; slot s occupies cols
    # [KC*offs[s], KC*offs[s+1]) as (k-chunk, token) row-major
    xt = nc.dram_tensor("xt", [128, KC * sumcap], f8, kind="ExternalInput").ap()
    # w: per (slot, dout-chunk) stationary tiles [128kk, KC*128mm]
    w = nc.dram_tensor("w", [EPC, MC, 128, KC * 128], f8,
                       kind="ExternalInput").ap()
    # out, m-major transposed pack: element (dout d, token t of slot s)
    # lives at [d % 128, (d//128)*sumcap + offs[s] + t]
    out = nc.dram_tensor("out", [128, MC * sumcap], bf16,
                         kind="ExternalOutput").ap()
    outv = out.rearrange("p (m t) -> p m t", m=MC)
    with tile.TileContext(nc) as tc:
        with (
            tc.tile_pool(name="xtp", bufs=1) as xt_pool,
            tc.tile_pool(name="wp", bufs=20) as w_pool,
            tc.tile_pool(name="op", bufs=10) as o_pool,
            tc.tile_pool(name="ps", bufs=2, space="PSUM") as ps_pool,
        ):
            # slot 0's x tiles arrive in 5-k-chunk groups on the sync queue
            # so its first matmuls fire early; the rest of x prefetches
            # whole-slot on the gpsimd queue during compute. The first
            # unit's weight tile is likewise split in 5-k pieces across
            # both rings so the very first matmul has ~80KB of deps.
            cap0 = int(caps[0])
            xt0 = {}
            w0 = {}
            for kg in range(4):
                xt0[kg] = xt_pool.tile([128, 5 * cap0], f8, tag=f"x0_{kg}",
                                       name=f"xt0_{kg}")
                nc.sync.dma_start(
                    xt0[kg][:],
                    xt[:, 5 * kg * cap0: 5 * (kg + 1) * cap0],
                )
                w0[kg] = w_pool.tile([128, 5 * 128], f8, tag=f"w0_{kg}",
                                     name=f"w0_{kg}")
                (nc.scalar if kg % 2 == 0 else nc.sync).dma_start(
                    w0[kg][:], w[0, 0, :, 5 * kg * 128:5 * (kg + 1) * 128]
                )
            xts = {}
            for s in range(1, EPC):
                cap = int(caps[s])
                off = int(offs[s])
                xts[s] = xt_pool.tile([128, KC * cap], f8, tag=f"xt{s}",
                                      name=f"xt{s}")
                nc.gpsimd.dma_start(
                    xts[s][:], xt[:, KC * off:KC * (off + cap)]
                )
            # round-robin (m-row major) over (slot, dout-chunk) units so the
            # per-slot weight stream (constant 4.26MB/slot) is spread evenly
            # against that slot's PE time; output flushes per m-pair.
            osb = {}
            u = 0
            for m in range(MC):
                for s in range(EPC):
                    cap = int(caps[s])
                    off = int(offs[s])
                    first = (u == 0)
                    if not first:
                        w_sb = w_pool.tile([128, KC * 128], f8, tag="w",
                                           name=f"w{s}_{m}")
                        # alternate the two HWDGE rings for the weight
                        # stream
                        weng = nc.scalar if u % 2 == 0 else nc.sync
                        weng.dma_start(w_sb[:], w[s, m])
                    ps = ps_pool.tile([128, cap], mybir.dt.float32,
                                      tag=f"ps{u % 4}", name=f"ps_{s}_{m}")
                    for k in range(KC):
                        rhs = (xt0[k // 5][:, (k % 5) * cap:(k % 5 + 1) * cap]
                               if s == 0 else
                               xts[s][:, k * cap:(k + 1) * cap])
                        lhsT = (w0[k // 5][:, (k % 5) * 128:(k % 5 + 1) * 128]
                                if first else
                                w_sb[:, k * 128:(k + 1) * 128])
                        nc.tensor.matmul(
                            ps[:],
                            lhsT,
                            rhs,
                            start=(k == 0),
                            stop=(k == KC - 1),
                        )
                    if m % 2 == 0 and m < MC - 1:
                        osb[s] = o_pool.tile([128, 2 * cap], bf16, tag="o",
                                             name=f"o_{s}_{m}")
                        nc.vector.tensor_copy(osb[s][:, :cap], ps[:])
                    else:
                        if m == MC - 1:
                            osb[s] = o_pool.tile([128, cap], bf16, tag="ol",
                                                 name=f"o_{s}_{m}")
                            nc.vector.tensor_copy(osb[s][:, :cap], ps[:])
                            npair = 1
                        else:
                            nc.vector.tensor_copy(osb[s][:, cap:], ps[:])
                            npair = 2
                        oeng = nc.sync if u % 2 == 0 else nc.scalar
                        oeng.dma_start(
                            outv[:, m + 1 - npair:m + 1, off:off + cap],
                            osb[s][:].rearrange("p (m t) -> p m t", m=npair),
                        )
                    u += 1
    nc.compile()
    return nc


def _run(inputs, trace=False):
    x = np.asarray(inputs["input"], dtype=np.float32)
    w = np.ascontiguousarray(np.asarray(inputs["weight"], dtype=np.float32))
    counts = np.asarray(inputs["tokens_per_expert"], dtype=np.int64)
    starts = np.concatenate([[0], np.cumsum(counts)[:-1]])

    order = np.argsort(-counts, kind="stable")  # experts by size rank
    # slot s, core c -> expert order[s*NCORES + c]; capacity = rank-group max
    caps = tuple(
        int(np.ceil(max(1, counts[order[s * NCORES:(s + 1) * NCORES]].max()) / 4) * 4)
        for s in range(EPC)
    )
    offs = np.concatenate([[0], np.cumsum(caps)]).astype(int)
    sumcap = int(offs[-1])

    if caps not in _cache:
        _cache[caps] = _build(caps)
    nc = _cache[caps]

    xs = x.astype(ml_dtypes.float8_e3m4)
    w8all = (w * WSCALE).astype(ml_dtypes.float8_e3m4)
    # stationary tiles: [DIN, DOUT] -> [MC, 128kk, KC*128mm]
    w8t = np.ascontiguousarray(
        w8all.reshape(G, KC, 128, MC, 128).transpose(0, 3, 2, 1, 4)
    ).reshape(G, MC, 128, KC * 128)

    in_maps = []
    for c in range(NCORES):
        xt_pack = np.zeros((128, KC * sumcap), dtype=ml_dtypes.float8_e3m4)
        w_pack = np.empty((EPC, MC, 128, KC * 128),
                          dtype=ml_dtypes.float8_e3m4)
        for s in range(EPC):
            g = int(order[s * NCORES + c])
            cnt = int(counts[g])
            cap = caps[s]
            if cnt:
                blk = np.zeros((128, KC, cap), dtype=ml_dtypes.float8_e3m4)
                blk[:, :, :cnt] = (
                    xs[starts[g]:starts[g] + cnt].T
                    .reshape(KC, 128, cnt).transpose(1, 0, 2)
                )
                xt_pack[:, KC * offs[s]:KC * offs[s + 1]] = \
                    blk.reshape(128, KC * cap)
            w_pack[s] = w8t[g]
        in_maps.append({"xt": xt_pack, "w": w_pack})

    kw = {"trace_cores": list(range(NCORES))} if trace else {}
    res = run_bass_kernel_spmd(nc, in_maps, core_ids=list(range(NCORES)),
                               trace=trace, **kw)

    out = np.empty((T, DOUT), dtype=np.float32)
    for c in range(NCORES):
        # [128, MC*sumcap] -> [MC, 128, sumcap] -> [DOUT, sumcap]
        o = np.asarray(res.results[c]["out"]).reshape(128, MC, sumcap) \
            .transpose(1, 0, 2).reshape(DOUT, sumcap)
        for s in range(EPC):
            g = int(order[s * NCORES + c])
            cnt = int(counts[g])
            if cnt:
                out[starts[g]:starts[g] + cnt] = \
                    o[:, offs[s]:offs[s] + cnt].T.astype(np.float32) * (1.0 / WSCALE)
    return out, res


def kernel(**inputs) -> np.ndarray:
    return _run(inputs)[0]
